# revision 4
# baseline (speedup 1.0000x reference)
"""TRN2 Bass kernel for nn_Cheb (ChebConv GNN, K=5, 3 layers + linear head).

Design:
  - Nodes dst-sharded across 8 cores (6250/core, padded to 6272=49*128).
  - Edge weights are separable: w = -dinv[src]*dinv[dst]; fold dinv[src]
    into a per-node prescale (u = dinv*T, bf16) and dinv[dst] into a
    per-column postscale. The segment-sum is then unweighted.
  - Per matvec: AllGather of u (bf16) -> per-core dma_gather of u[src]
    rows (256B each) in dst-window order -> selection-matrix matmuls
    accumulate each 128-edge tile into a [feat, 128-dst] PSUM tile
    (order-robust, so the int16 gather-index limit is handled by
    splitting sources at 32768) -> feat-major Chebyshev recurrence,
    dense W_k matmuls, PE transposes to produce the next u.
"""
import os
import numpy as np
import ml_dtypes

import concourse.bass as bass
import concourse.mybir as mybir
import concourse.tile as tile
from concourse import bacc
from concourse.bass_utils import run_bass_kernel_spmd

bf16 = ml_dtypes.bfloat16

M = 8
N = 50000
F = 128
K = 5
SHARD = 6250
NB = 49                 # 128-node blocks per shard
SP = NB * 128           # 6272 padded shard
NPAD = M * SP           # 50176
NW = NB                 # dst windows per core
HI = 32768              # int16 gather index limit
GW = 2                  # windows per gather call
MAXT = 36               # max tiles per gather call

f32 = mybir.dt.float32
bt = mybir.dt.bfloat16
i16 = mybir.dt.int16

LAST_EXEC_NS = None
LAST_RESULTS = None


class Sched:
    pass


def _host_prep(edge_index):
    src = edge_index[0].astype(np.int64)
    dst = edge_index[1].astype(np.int64)
    ew = src != dst
    deg = np.bincount(src[ew], minlength=N)
    dinv = np.where(deg > 0, 1.0 / np.sqrt(np.maximum(deg, 1.0)), 0.0).astype(np.float32)
    keep = ew & (dinv[src] > 0) & (dinv[dst] > 0)
    es, ed = src[keep], dst[keep]
    own = ed // SHARD
    ps = (es // SHARD) * SP + (es % SHARD)     # padded global src id
    dl = ed - own * SHARD                      # local dst

    # per-core per-(window, pass) edge lists
    percore = []
    for c in range(M):
        m = own == c
        s_c, d_c = ps[m], dl[m]
        w_c = d_c // 128
        cells = {}
        for w in range(NW):
            mw = w_c == w
            cells[w] = (s_c[mw], d_c[mw] - w * 128)
        percore.append(cells)

    ntiles = {}
    for w in range(NW):
        ntiles[w] = max(1, max((len(percore[c][w][0]) + 127) // 128
                               for c in range(M)))

    groups = [list(range(g, min(g + GW, NW))) for g in range(0, NW, GW)]
    tiles = []   # [window, start, stop]
    for ws in groups:
        for w in ws:
            for i in range(ntiles[w]):
                tiles.append([w, i == 0, i == ntiles[w] - 1])
    T = len(tiles)

    # per-core packed tile data: int32 gather indices + bf16 dst columns
    gidx_all, dcol_all = [], []
    for c in range(M):
        idx_full = np.zeros(T * 128, np.int64)
        dcl_full = np.full(T * 128, -1.0, np.float32)
        tt = 0
        for ws in groups:
            for w in ws:
                nt = ntiles[w]
                ss, dc = percore[c][w]
                sl = slice(tt * 128, tt * 128 + len(ss))
                idx_full[sl] = ss
                dcl_full[sl] = dc
                tt += nt
        assert tt == T
        pos = np.arange(T * 128)
        gidx = np.zeros((128, T), np.int32)
        gidx[pos % 128, pos // 128] = idx_full
        dcol = np.zeros((128, T), bf16)
        dcol[pos % 128, pos // 128] = dcl_full.astype(bf16)
        gidx_all.append(gidx)
        dcol_all.append(dcol)

    dinv_pad = np.zeros((M, SP), np.float32)
    for c in range(M):
        dinv_pad[c, :SHARD] = dinv[c * SHARD:(c + 1) * SHARD]

    s = Sched()
    s.tiles, s.groups, s.T = tiles, groups, T
    s.gidx_all, s.dcol_all, s.dinv_pad = gidx_all, dcol_all, dinv_pad
    return s


def _build(s, bl_val, sim=False):
    nc = bacc.Bacc("TRN2", target_bir_lowering=False, debug=False,
                   enable_asserts=False, num_devices=1 if sim else M)
    T = s.T
    x_t = nc.dram_tensor("x_shard", [SP, F], f32, kind="ExternalInput")
    dinvcol_t = nc.dram_tensor("dinv_col", [128, NB], f32, kind="ExternalInput")
    dinvm2_t = nc.dram_tensor("dinvm2", [128, SP], f32, kind="ExternalInput")
    dcol_t = nc.dram_tensor("dst_cols", [128, T], bt, kind="ExternalInput")
    gidx_t = nc.dram_tensor("gidx", [128, T], mybir.dt.int32, kind="ExternalInput")
    iota_t = nc.dram_tensor("iota", [128, 128], bt, kind="ExternalInput")
    ident_t = nc.dram_tensor("ident", [128, 128], f32, kind="ExternalInput")
    wall_t = nc.dram_tensor("wall", [3 * K * 128, 128], f32, kind="ExternalInput")
    wl_t = nc.dram_tensor("wl", [128, 1], f32, kind="ExternalInput")
    bias_t = nc.dram_tensor("bias", [128, 3], f32, kind="ExternalInput")
    y_t = nc.dram_tensor("y_shard", [SP, 1], f32, kind="ExternalOutput")

    with tile.TileContext(nc) as tc:
        with (
            tc.tile_pool(name="persist", bufs=1) as pp,
            tc.tile_pool(name="mb", bufs=8) as mpool,
            tc.tile_pool(name="sel", bufs=8) as spool,
            tc.tile_pool(name="pw", bufs=4, space="PSUM") as ppool,
            tc.tile_pool(name="pd", bufs=2, space="PSUM") as dpool,
            tc.tile_pool(name="pt", bufs=2, space="PSUM") as tpool,
            tc.tile_pool(name="dram", bufs=1, space="DRAM") as dram,
        ):
            TA = pp.tile([128, SP], f32, tag="TA")
            TB = pp.tile([128, SP], f32, tag="TB")
            sT = pp.tile([128, SP], f32, tag="sT")
            outacc = pp.tile([128, SP], f32, tag="outacc")
            dinvm2 = pp.tile([128, SP], f32, tag="dinvm2")
            u_sb = pp.tile([128, SP], bt, tag="u")
            gidx_sb = pp.tile([128, T], mybir.dt.int32, tag="gidx")
            dcol_sb = pp.tile([128, T], bt, tag="dcol")
            iota_sb = pp.tile([128, 128], bt, tag="iota")
            ident_sb = pp.tile([128, 128], f32, tag="ident")
            wsb = pp.tile([128, 3 * K * 128], f32, tag="wsb")
            wl_sb = pp.tile([128, 1], f32, tag="wl")
            b_sb = pp.tile([128, 3], f32, tag="b")
            dinvcol_sb = pp.tile([128, NB], f32, tag="dinvcol")
            y_sb = pp.tile([128, NB], f32, tag="ysb")
            xw = pp.tile([128, SP], f32, tag="xw")

            u_bounce = dram.tile([SP, F], bt)

            # ---- setup loads ----
            nc.sync.dma_start(out=xw[:].rearrange("p (b f) -> p b f", f=128),
                              in_=x_t.ap().rearrange("(b p) f -> p b f", p=128))
            nc.sync.dma_start(out=dinvm2[:], in_=dinvm2_t.ap())
            nc.sync.dma_start(out=gidx_sb[:], in_=gidx_t.ap())
            nc.sync.dma_start(out=dcol_sb[:], in_=dcol_t.ap())
            nc.sync.dma_start(out=iota_sb[:], in_=iota_t.ap())
            nc.sync.dma_start(out=ident_sb[:], in_=ident_t.ap())
            nc.sync.dma_start(out=wsb[:].rearrange("p (i w) -> p i w", w=128),
                              in_=wall_t.ap().rearrange("(i p) w -> p i w", p=128))
            nc.sync.dma_start(out=wl_sb[:], in_=wl_t.ap())
            nc.sync.dma_start(out=b_sb[:], in_=bias_t.ap())
            nc.sync.dma_start(out=dinvcol_sb[:], in_=dinvcol_t.ap())

            def prod_u(Tx):
                # feat-major Tx -> node-major u = dinv*Tx (bf16)
                for b in range(NB):
                    pt = tpool.tile([128, 128], f32, tag="pt")
                    nc.tensor.transpose(pt[:], Tx[:, b * 128:(b + 1) * 128], ident_sb[:])
                    nc.vector.tensor_scalar_mul(
                        out=u_sb[:, b * 128:(b + 1) * 128], in0=pt[:],
                        scalar1=dinvcol_sb[:, b:b + 1])

            def matvec():
                # exchange u, gather, segment-sum into sT (feat-major)
                u_all = dram.tile([NPAD, F], bt, addr_space="Shared",
                                  tag="uall", bufs=12, name="uall")
                nc.sync.dma_start(
                    out=u_bounce[:].rearrange("(b p) f -> p b f", p=128),
                    in_=u_sb[:].rearrange("p (b f) -> p b f", f=128))
                if sim:
                    # local stand-in for the AllGather: write all 8 shard
                    # slots from the local bounce buffer (same HBM write
                    # traffic as the real collective's receive path)
                    for c in range(M):
                        nc.sync.dma_start(out=u_all[c * SP:(c + 1) * SP, :],
                                          in_=u_bounce[:])
                else:
                    nc.gpsimd.collective_compute(
                        "AllGather", mybir.AluOpType.bypass,
                        replica_groups=[list(range(M))],
                        ins=[u_bounce.opt()], outs=[u_all.opt()])
                t = 0
                for ws in s.groups:
                    pw = {}
                    t0 = t
                    while t < len(s.tiles) and s.tiles[t][0] in ws:
                        w, st, sp_ = s.tiles[t]
                        Mb = mpool.tile([128, 128], bt, tag="mb")
                        nc.gpsimd.indirect_dma_start(
                            out=Mb[:], out_offset=None,
                            in_=u_all[:],
                            in_offset=bass.IndirectOffsetOnAxis(
                                ap=gidx_sb[:, t:t + 1], axis=0))
                        sel = spool.tile([128, 128], bt, tag="sel")
                        nc.vector.tensor_tensor(
                            out=sel[:],
                            in0=dcol_sb[:, t:t + 1].to_broadcast([128, 128]),
                            in1=iota_sb[:], op=mybir.AluOpType.is_equal)
                        if st:
                            pw[w] = ppool.tile([128, 128], f32, tag="pw", name="pw")
                        nc.tensor.matmul(pw[w][:], lhsT=Mb[:], rhs=sel[:],
                                         start=st, stop=sp_)
                        t += 1
                    for w in ws:
                        nc.scalar.activation(
                            out=sT[:, w * 128:(w + 1) * 128], in_=pw[w][:],
                            func=mybir.ActivationFunctionType.Copy)

            CH = [(c, min(512, SP - c)) for c in range(0, SP, 512)]

            def dense_acc(l, k, Tx, first):
                wi = l * K + k
                for c0, cn in CH:
                    pd = dpool.tile([128, 512], f32, tag="pd")
                    nc.tensor.matmul(pd[:, :cn],
                                     lhsT=wsb[:, wi * 128:(wi + 1) * 128],
                                     rhs=Tx[:, c0:c0 + cn], start=True, stop=True)
                    if first:
                        nc.scalar.activation(out=outacc[:, c0:c0 + cn], in_=pd[:, :cn],
                                             func=mybir.ActivationFunctionType.Copy)
                    else:
                        nc.vector.tensor_tensor(out=outacc[:, c0:c0 + cn],
                                                in0=outacc[:, c0:c0 + cn],
                                                in1=pd[:, :cn], op=mybir.AluOpType.add)

            # ---- T0 = x^T, u0 = dinv*x ----
            for b in range(NB):
                pt = tpool.tile([128, 128], f32, tag="pt")
                nc.tensor.transpose(pt[:], xw[:, b * 128:(b + 1) * 128], ident_sb[:])
                nc.scalar.activation(out=TA[:, b * 128:(b + 1) * 128], in_=pt[:],
                                     func=mybir.ActivationFunctionType.Copy)
                nc.vector.tensor_scalar_mul(
                    out=u_sb[:, b * 128:(b + 1) * 128],
                    in0=xw[:, b * 128:(b + 1) * 128],
                    scalar1=dinvcol_sb[:, b:b + 1])

            A, B = TA, TB
            for l in range(3):
                dense_acc(l, 0, A, first=True)
                for k in range(1, K):
                    matvec()
                    # m = sT * (-2*dinv) per column
                    nc.vector.tensor_tensor(out=sT[:], in0=sT[:], in1=dinvm2[:],
                                            op=mybir.AluOpType.mult)
                    if k == 1:
                        nc.scalar.activation(out=B[:], in_=sT[:],
                                             func=mybir.ActivationFunctionType.Copy,
                                             scale=0.5)
                        Tx = B
                    else:
                        dst = A if (k % 2 == 0) else B
                        nc.vector.tensor_tensor(out=dst[:], in0=sT[:], in1=dst[:],
                                                op=mybir.AluOpType.subtract)
                        Tx = dst
                    dense_acc(l, k, Tx, first=False)
                    if k < K - 1:
                        prod_u(Tx)
                # layer output (after T4 in A): h -> B
                if l < 2:
                    nc.scalar.activation(out=B[:], in_=outacc[:],
                                         func=mybir.ActivationFunctionType.Relu,
                                         bias=b_sb[:, l:l + 1])
                    prod_u(B)
                else:
                    nc.vector.tensor_scalar_add(out=B[:], in0=outacc[:],
                                                scalar1=b_sb[:, 2:3])
                A, B = B, A

            # ---- head: y = h @ Wl + bl ----
            h3 = A
            for b in range(NB):
                ph = tpool.tile([128, 128], f32, tag="pt")
                nc.tensor.matmul(ph[:, :1], lhsT=h3[:, b * 128:(b + 1) * 128],
                                 rhs=wl_sb[:], start=True, stop=True)
                nc.scalar.activation(out=y_sb[:, b:b + 1], in_=ph[:, :1],
                                     func=mybir.ActivationFunctionType.Copy,
                                     bias=float(bl_val))
            nc.sync.dma_start(out=y_t.ap().rearrange("(b p) o -> p b o", p=128),
                              in_=y_sb[:].rearrange("p b -> p b ()"))
    nc.compile()
    return nc


def make_in_maps(s, inputs):
    x = np.asarray(inputs['x'], np.float32)
    wall = np.concatenate([np.asarray(inputs[f'W{l}'], np.float32).reshape(K * 128, 128)
                           for l in range(3)], axis=0)
    bias = np.stack([np.asarray(inputs[f'b{l}'], np.float32) for l in range(3)], axis=1)
    iota = np.broadcast_to(np.arange(128, dtype=bf16)[None, :], (128, 128)).copy()
    ident = np.eye(128, dtype=np.float32)
    wl = np.asarray(inputs['Wl'], np.float32)

    in_maps = []
    for c in range(M):
        xs = np.zeros((SP, F), np.float32)
        xs[:SHARD] = x[c * SHARD:(c + 1) * SHARD]
        dp = s.dinv_pad[c]
        dinvcol = dp.reshape(NB, 128).T.copy()
        dinvm2 = np.broadcast_to((-2.0 * dp)[None, :], (128, SP)).copy()
        in_maps.append({
            'x_shard': xs, 'dinv_col': np.ascontiguousarray(dinvcol),
            'dinvm2': dinvm2, 'dst_cols': s.dcol_all[c], 'gidx': s.gidx_all[c],
            'iota': iota, 'ident': ident, 'wall': wall, 'wl': wl, 'bias': bias,
        })
    return in_maps


def kernel(**inputs):
    global LAST_EXEC_NS, LAST_RESULTS
    edge_index = np.asarray(inputs['edge_index'])
    s = _host_prep(edge_index)
    bl_val = float(np.asarray(inputs['bl']).reshape(-1)[0])
    nc = _build(s, bl_val)

    in_maps = make_in_maps(s, inputs)
    res = run_bass_kernel_spmd(nc, in_maps, core_ids=list(range(M)),
                               trace=bool(int(os.environ.get("KTRACE", "0"))))
    LAST_EXEC_NS = res.exec_time_ns
    LAST_RESULTS = res
    y = np.zeros((N, 1), np.float32)
    for c in range(M):
        y[c * SHARD:(c + 1) * SHARD] = res.results[c]['y_shard'][:SHARD]
    return y

